# revision 27
# baseline (speedup 1.0000x reference)
# Fused GEMM + GroupNorm + swish*mw + swish kernel for 8 Trainium2 cores.
#
# reference math (per full problem):
#   y  = x @ W^T + b                      [M, N] = [16384, 4096]
#   yn = GroupNorm(y, groups=256)         (group size 16 along N, eps=1e-6)
#   yn = yn * gn_weight + gn_bias
#   z  = swish(yn) * multiply_weight
#   out= swish(z)
#
# Sharding: data-parallel along M. Each of the 8 cores gets M/8 = 2048 rows of
# x and the full weight/params; outputs are concatenated along M.  The x-shard
# and weight are cast fp32->bf16 host-side (input layout prep); bias is kept
# fp32 on device, PSUM accumulation is fp32, so accuracy matches an
# fp32-accumulated bf16 GEMM.
#
# Per-core schedule:
#   - GEMM operands are loaded K-major via HW xbar DMA-transpose (bf16-only
#     path) on the sync-engine HWDGE queue.  (All transposes stay on ONE
#     queue: concurrent transposes from both HWDGE rings corrupt data on
#     TRN2.)  x^T for the whole 2048-row shard stays resident in SBUF
#     (128KB/partition), so the weight is transpose-read exactly once.
#   - PE: out-tile [128m, 512n] accumulates 32 k-matmuls (bf16 x bf16 -> fp32
#     PSUM) plus one K=1 fp32r matmul of ones^T @ bias_slice that adds the
#     bias at full precision.  The k loop runs in two half-K waves across a
#     group of 8 m-subtiles so each wT half-buffer frees early for prefetch.
#   - GroupNorm stats are reduced straight out of PSUM (DVE sum, ACT square +
#     DVE sum); rstd uses a Quake-style rsqrt (bit trick + 2 Newton steps) on
#     DVE, batched over 4 m-subtiles, avoiding ACT table swaps.
#   - normalize on DVE (scalar_tensor_tensor), mean-sub + multiply_weight on
#     GpSimd, swish = x*sigmoid(x) with Sigmoid on ACT and multiplies on DVE,
#     output stores on the GpSimd (SWDGE) queue.

import numpy as np

P = 128
GS = 16  # group size = N / NUM_GROUPS = 4096 / 256
EPS = 1e-6

M_FULL, K_FULL, N_FULL = 16384, 4096, 4096
N_CORES = 8


def build_nc(M_SHARD, K, N, apply_affine, n_tile=512, m_blk=2048,
             split_queues=False, kt_outer=True):
    import concourse.bass as bass
    import concourse.tile as tile
    from concourse import bacc, mybir

    f32 = mybir.dt.float32
    f32r = mybir.dt.float32r
    bf16 = mybir.dt.bfloat16
    i32 = mybir.dt.int32
    Alu = mybir.AluOpType
    Act = mybir.ActivationFunctionType
    X = mybir.AxisListType.X

    KT = K // P                    # k-tiles of 128
    KH = max(KT // 2, 1)           # k-tiles per wT half-buffer
    NKH = KT // KH                 # number of k-half waves (2)
    N_TILES = N // n_tile
    NG = n_tile // GS              # groups per n-tile
    m_blk = min(m_blk, M_SHARD)
    M_BLKS = M_SHARD // m_blk
    MS_PER_BLK = m_blk // P
    SGRP = min(8, MS_PER_BLK)      # m-subtiles per PSUM wave group
    SG = min(4, SGRP)              # m-subtiles per stats batch

    nc = bacc.Bacc("TRN2", target_bir_lowering=False)

    x = nc.dram_tensor("x", [M_SHARD, K], bf16, kind="ExternalInput")
    w = nc.dram_tensor("weight", [N, K], bf16, kind="ExternalInput")
    bias = nc.dram_tensor("bias", [N], f32, kind="ExternalInput")
    mw = nc.dram_tensor("multiply_weight", [N], f32, kind="ExternalInput")
    if apply_affine:
        gnw = nc.dram_tensor("gn_weight", [N], f32, kind="ExternalInput")
        gnb = nc.dram_tensor("gn_bias", [N], f32, kind="ExternalInput")
    out = nc.dram_tensor("out", [M_SHARD, N], f32, kind="ExternalOutput")

    def bcast_rows(ap_1d, rows):
        # DRAM [n] -> broadcast-read AP [[0, rows], [1, n]]
        return bass.AP(ap_1d.tensor, ap_1d.offset, [[0, rows]] + list(ap_1d.ap))

    with tile.TileContext(nc) as tc:
        from contextlib import ExitStack

        with ExitStack() as ctx:
            xT_pool = ctx.enter_context(
                tc.tile_pool(name="xT", bufs=(1 if M_BLKS == 1 else 2))
            )
            wT_pool = ctx.enter_context(tc.tile_pool(name="wT", bufs=3))
            psum_pool = ctx.enter_context(
                tc.tile_pool(name="psum", bufs=8, space="PSUM")
            )
            big_pool = ctx.enter_context(tc.tile_pool(name="big", bufs=7))
            o_pool = ctx.enter_context(tc.tile_pool(name="o", bufs=2))
            stats_pool = ctx.enter_context(tc.tile_pool(name="stats", bufs=2))
            small_pool = ctx.enter_context(tc.tile_pool(name="small", bufs=2))
            param_pool = ctx.enter_context(tc.tile_pool(name="param", bufs=2))
            const_pool = ctx.enter_context(tc.tile_pool(name="const", bufs=1))

            # ---- constants ----
            magic_f = const_pool.tile([P, 1], f32)
            nc.vector.memset(
                magic_f,
                float(np.frombuffer(np.uint32(0x5F3759DF).tobytes(), np.float32)[0]),
            )
            magic = magic_f.bitcast(i32)
            # fp32r operands keep the bias add at full fp32 precision while
            # running at 1 cycle/row (moving dim 512 >= 256).
            ones_row_f = const_pool.tile([1, P], f32)
            nc.vector.memset(ones_row_f, 1.0)
            ones_row = ones_row_f.bitcast(f32r)

            hwdge = [nc.sync, nc.scalar] if split_queues else [nc.sync, nc.sync]

            # ---- GEMM + epilogue ----
            for mb in range(M_BLKS):
                mrow0 = mb * m_blk
                # x^T tile [P(k), KT, m_blk] -- fully resident
                xT = xT_pool.tile([P, KT, m_blk], bf16, tag="xT")

                def load_xT(kt):
                    hwdge[kt % 2].dma_start_transpose(
                        xT[:, kt, :],
                        x[mrow0 : mrow0 + m_blk, kt * P : (kt + 1) * P],
                    )

                if mb > 0:
                    for kt in range(KT):
                        load_xT(kt)

                for nt in range(N_TILES):
                    ncol0 = nt * n_tile
                    # wT k-half tiles: [P(k), KH, n_tile]
                    whs = []
                    for h in range(NKH):
                        wT = wT_pool.tile([P, KH, n_tile], bf16, tag="wT")
                        for j in range(KH):
                            kt = h * KH + j
                            hwdge[j % 2].dma_start_transpose(
                                wT[:, j, :],
                                w[ncol0 : ncol0 + n_tile, kt * P : (kt + 1) * P],
                            )
                            if mb == 0 and nt == 0:
                                # interleave x^T loads with the first n-tile's
                                # wT loads so the first matmuls unblock early
                                load_xT(kt)
                        whs.append(wT)

                    # per-channel params for this n-tile
                    bias_sb = param_pool.tile([1, n_tile], f32r, tag="bias_sb")
                    nc.gpsimd.dma_start(
                        out=bias_sb, in_=bcast_rows(bias[ncol0 : ncol0 + n_tile], 1)
                    )
                    mw_rep = param_pool.tile([P, n_tile], f32, tag="mw_rep")
                    nc.gpsimd.dma_start(
                        out=mw_rep, in_=bcast_rows(mw[ncol0 : ncol0 + n_tile], P)
                    )
                    if apply_affine:
                        gnw_rep = param_pool.tile([P, n_tile], f32, tag="gnw_rep")
                        nc.gpsimd.dma_start(
                            out=gnw_rep, in_=bcast_rows(gnw[ncol0 : ncol0 + n_tile], P)
                        )
                        gnb_rep = param_pool.tile([P, n_tile], f32, tag="gnb_rep")
                        nc.gpsimd.dma_start(
                            out=gnb_rep, in_=bcast_rows(gnb[ncol0 : ncol0 + n_tile], P)
                        )

                    for grp0 in range(0, MS_PER_BLK, SGRP):
                        ms_list = list(range(grp0, min(grp0 + SGRP, MS_PER_BLK)))
                        pss = {}
                        for ms in ms_list:
                            pss[ms] = psum_pool.tile(
                                [P, n_tile], f32, tag="ps", name=f"ps{ms}"
                            )

                        # k-half waves: all subtiles do kh0, then kh1 --
                        # frees the kh0 wT buffer at ~75% of the n-tile.
                        def emit_mms(ms, h):
                            moff = ms * P
                            for j in range(KH):
                                kt = h * KH + j
                                nc.tensor.matmul(
                                    pss[ms],
                                    lhsT=xT[:, kt, moff : moff + P],
                                    rhs=whs[h][:, j, :],
                                    start=(kt == 0),
                                    stop=False,
                                )
                            if h == NKH - 1:
                                # += ones^T @ bias (adds bias to every row)
                                nc.tensor.matmul(
                                    pss[ms],
                                    lhsT=ones_row[0:1, :],
                                    rhs=bias_sb[0:1, :],
                                    start=False,
                                    stop=True,
                                )

                        if kt_outer:
                            for h in range(NKH):
                                for ms in ms_list:
                                    emit_mms(ms, h)
                        else:
                            for ms in ms_list:
                                for h in range(NKH):
                                    emit_mms(ms, h)

                        for sg0 in range(0, len(ms_list), SG):
                            msl_list = ms_list[sg0 : sg0 + SG]
                            nsg = len(msl_list)
                            sums = stats_pool.tile([P, SG, NG], f32, tag="sums")
                            sqs = stats_pool.tile([P, SG, NG], f32, tag="sqs")
                            t0s = {}
                            for i, ms in enumerate(msl_list):
                                ps = pss[ms]
                                # evacuate PSUM immediately on ACT (frees the
                                # bank for the next wave group's matmuls)
                                t0 = big_pool.tile([P, n_tile], f32, tag="big")
                                nc.scalar.copy(t0, ps)
                                t0s[ms] = t0
                                nc.vector.reduce_sum(
                                    sums[:, i, :],
                                    t0.rearrange("p (g s) -> p g s", s=GS),
                                    axis=X,
                                )
                                sq = big_pool.tile([P, n_tile], f32, tag="big")
                                nc.scalar.square(sq, t0)
                                nc.vector.reduce_sum(
                                    sqs[:, i, :],
                                    sq.rearrange("p (g s) -> p g s", s=GS),
                                    axis=X,
                                )

                            # batched small stats over [P, nsg*NG]
                            sums_f = sums[:, :nsg, :].rearrange("p a b -> p (a b)")
                            sqs_f = sqs[:, :nsg, :].rearrange("p a b -> p (a b)")
                            nb = nsg * NG
                            m2 = small_pool.tile(
                                [P, SG * NG], f32, tag="m2", name="m2"
                            )[:, :nb]
                            nc.vector.tensor_mul(m2, sums_f, sums_f)
                            u = small_pool.tile([P, SG * NG], f32, tag="u", name="u")[
                                :, :nb
                            ]
                            # u = GS*sum(y^2) - sum(y)^2 = GS^2 * var
                            nc.vector.scalar_tensor_tensor(
                                out=u,
                                in0=sqs_f,
                                scalar=float(GS),
                                in1=m2,
                                op0=Alu.mult,
                                op1=Alu.subtract,
                            )
                            nc.vector.tensor_scalar(
                                out=u,
                                in0=u,
                                scalar1=float(GS * GS) * EPS,
                                scalar2=None,
                                op0=Alu.add,
                            )
                            # r = rsqrt(u) = rstd / GS  (Quake + 2 Newton steps)
                            rt = small_pool.tile([P, SG, NG], f32, tag="rt")
                            r = rt[:, :nsg, :].rearrange("p a b -> p (a b)")
                            nc.vector.tensor_scalar(
                                out=r.bitcast(i32),
                                in0=u.bitcast(i32),
                                scalar1=1,
                                scalar2=None,
                                op0=Alu.arith_shift_right,
                            )
                            nc.vector.tensor_tensor(
                                out=r.bitcast(i32),
                                in0=magic.broadcast_to([P, nb]),
                                in1=r.bitcast(i32),
                                op=Alu.subtract,
                            )
                            tnr = small_pool.tile(
                                [P, SG * NG], f32, tag="m2", name="tnr"
                            )[:, :nb]
                            for _ in range(2):
                                nc.vector.tensor_mul(tnr, r, r)
                                nc.vector.tensor_mul(tnr, tnr, u)
                                nc.vector.tensor_scalar(
                                    out=tnr,
                                    in0=tnr,
                                    scalar1=-0.5,
                                    scalar2=1.5,
                                    op0=Alu.mult,
                                    op1=Alu.add,
                                )
                                nc.vector.tensor_mul(r, r, tnr)
                            # U = sum(y) * r = mean * rstd
                            Ut = small_pool.tile([P, SG, NG], f32, tag="m2", name="Ut")
                            nc.vector.tensor_mul(
                                Ut[:, :nsg, :].rearrange("p a b -> p (a b)"),
                                sums_f,
                                r,
                            )

                            for i, ms in enumerate(msl_list):
                                t0 = t0s[ms]
                                t03 = t0.rearrange("p (g s) -> p g s", s=GS)
                                rb = bass.AP(
                                    rt.tensor,
                                    rt[:, i, :].offset,
                                    list(rt[:, i, :].ap) + [[0, GS]],
                                )
                                ub = bass.AP(
                                    Ut.tensor,
                                    Ut[:, i, :].offset,
                                    list(Ut[:, i, :].ap) + [[0, GS]],
                                )
                                t1 = big_pool.tile([P, n_tile], f32, tag="big")
                                t13 = t1.rearrange("p (g s) -> p g s", s=GS)
                                # t1 = (t0 * GS) * r = t0 * rstd
                                nc.vector.scalar_tensor_tensor(
                                    out=t13,
                                    in0=t03,
                                    scalar=float(GS),
                                    in1=rb,
                                    op0=Alu.mult,
                                    op1=Alu.mult,
                                )
                                # t1 -= mean * rstd
                                nc.gpsimd.tensor_tensor(
                                    out=t13, in0=t13, in1=ub, op=Alu.subtract
                                )
                                if apply_affine:
                                    nc.gpsimd.tensor_mul(t1, t1, gnw_rep)
                                    nc.gpsimd.tensor_add(t1, t1, gnb_rep)
                                # swish(v) = v * sigmoid(v); Sigmoid + Square
                                # share one ACT table set (no reloads).
                                s1 = big_pool.tile([P, n_tile], f32, tag="big")
                                nc.scalar.activation(s1, t1, Act.Sigmoid)
                                z = big_pool.tile([P, n_tile], f32, tag="big")
                                nc.vector.tensor_mul(z, s1, t1)
                                nc.gpsimd.tensor_mul(z, z, mw_rep)
                                s2 = big_pool.tile([P, n_tile], f32, tag="big")
                                nc.scalar.activation(s2, z, Act.Sigmoid)
                                o = o_pool.tile([P, n_tile], f32, tag="oo")
                                nc.vector.tensor_mul(o, z, s2)
                                r0 = mrow0 + ms * P
                                nc.gpsimd.dma_start(
                                    out=out[r0 : r0 + P, ncol0 : ncol0 + n_tile],
                                    in_=o,
                                )

    nc.compile()
    return nc


_CACHE = {}


def _get_nc(M_SHARD, K, N, apply_affine):
    key = (M_SHARD, K, N, apply_affine)
    if key not in _CACHE:
        _CACHE[key] = build_nc(M_SHARD, K, N, apply_affine)
    return _CACHE[key]


def make_in_maps(inputs, apply_affine=None):
    import ml_dtypes

    x = np.ascontiguousarray(inputs["x"], dtype=np.float32)
    w = np.ascontiguousarray(inputs["weight"], dtype=np.float32)
    bias = np.ascontiguousarray(inputs["bias"], dtype=np.float32)
    mw = np.ascontiguousarray(inputs["multiply_weight"], dtype=np.float32)
    gnw = np.ascontiguousarray(inputs["gn_weight"], dtype=np.float32)
    gnb = np.ascontiguousarray(inputs["gn_bias"], dtype=np.float32)

    M, K = x.shape
    N = w.shape[0]
    M_SHARD = M // N_CORES
    if apply_affine is None:
        apply_affine = not (np.all(gnw == 1.0) and np.all(gnb == 0.0))

    xb = x.astype(ml_dtypes.bfloat16)
    wb = np.ascontiguousarray(w.astype(ml_dtypes.bfloat16))

    in_maps = []
    for c in range(N_CORES):
        m = {
            "x": np.ascontiguousarray(xb[c * M_SHARD : (c + 1) * M_SHARD]),
            "weight": wb,
            "bias": bias,
            "multiply_weight": mw,
        }
        if apply_affine:
            m["gn_weight"] = gnw
            m["gn_bias"] = gnb
        in_maps.append(m)
    return in_maps, M_SHARD, K, N, apply_affine


def kernel(**inputs):
    from concourse.bass_utils import run_bass_kernel_spmd

    in_maps, M_SHARD, K, N, apply_affine = make_in_maps(inputs)
    nc = _get_nc(M_SHARD, K, N, apply_affine)
    res = run_bass_kernel_spmd(nc, in_maps, core_ids=list(range(N_CORES)))
    return np.concatenate([r["out"] for r in res.results], axis=0)


if __name__ == "__main__":
    import reference

    inputs = {k: np.asarray(v) for k, v in reference.setup_inputs().items()}
    out = kernel(**inputs)
    print(out.shape, out.dtype)


# revision 28
# speedup vs baseline: 1.0188x; 1.0188x over previous
# Fused GEMM + GroupNorm + swish*mw + swish kernel for 8 Trainium2 cores.
#
# reference math (per full problem):
#   y  = x @ W^T + b                      [M, N] = [16384, 4096]
#   yn = GroupNorm(y, groups=256)         (group size 16 along N, eps=1e-6)
#   yn = yn * gn_weight + gn_bias
#   z  = swish(yn) * multiply_weight
#   out= swish(z)
#
# Sharding: data-parallel along M. Each of the 8 cores gets M/8 = 2048 rows of
# x and the full weight/params; outputs are concatenated along M.  The x-shard
# and weight are cast fp32->bf16 host-side (input layout prep); bias is kept
# fp32 on device, PSUM accumulation is fp32, so accuracy matches an
# fp32-accumulated bf16 GEMM.
#
# Per-core schedule:
#   - GEMM operands are loaded K-major via HW xbar DMA-transpose (bf16-only
#     path) on the sync-engine HWDGE queue.  (All transposes stay on ONE
#     queue: concurrent transposes from both HWDGE rings corrupt data on
#     TRN2.)  x^T for the whole 2048-row shard stays resident in SBUF
#     (128KB/partition), so the weight is transpose-read exactly once.
#   - PE: out-tile [128m, 512n] accumulates 32 k-matmuls (bf16 x bf16 -> fp32
#     PSUM) plus one K=1 fp32r matmul of ones^T @ bias_slice that adds the
#     bias at full precision.  The k loop runs in two half-K waves across a
#     group of 8 m-subtiles so each wT half-buffer frees early for prefetch.
#   - GroupNorm stats are reduced straight out of PSUM (DVE sum, ACT square +
#     DVE sum); rstd uses a Quake-style rsqrt (bit trick + 2 Newton steps) on
#     DVE, batched over 4 m-subtiles, avoiding ACT table swaps.
#   - normalize on DVE (scalar_tensor_tensor), mean-sub + multiply_weight on
#     GpSimd, swish = x*sigmoid(x) with Sigmoid on ACT and multiplies on DVE,
#     output stores on the GpSimd (SWDGE) queue.

import numpy as np

P = 128
GS = 16  # group size = N / NUM_GROUPS = 4096 / 256
EPS = 1e-6

M_FULL, K_FULL, N_FULL = 16384, 4096, 4096
N_CORES = 8


def build_nc(M_SHARD, K, N, apply_affine, n_tile=512, m_blk=2048,
             split_queues=False, kt_outer=True):
    import concourse.bass as bass
    import concourse.tile as tile
    from concourse import bacc, mybir

    f32 = mybir.dt.float32
    f32r = mybir.dt.float32r
    bf16 = mybir.dt.bfloat16
    i32 = mybir.dt.int32
    Alu = mybir.AluOpType
    Act = mybir.ActivationFunctionType
    X = mybir.AxisListType.X

    KT = K // P                    # k-tiles of 128
    KH = max(KT // 2, 1)           # k-tiles per wT half-buffer
    NKH = KT // KH                 # number of k-half waves (2)
    N_TILES = N // n_tile
    NG = n_tile // GS              # groups per n-tile
    m_blk = min(m_blk, M_SHARD)
    M_BLKS = M_SHARD // m_blk
    MS_PER_BLK = m_blk // P
    SGRP = min(8, MS_PER_BLK)      # m-subtiles per PSUM wave group
    SG = min(4, SGRP)              # m-subtiles per stats batch

    nc = bacc.Bacc("TRN2", target_bir_lowering=False)

    x = nc.dram_tensor("x", [M_SHARD, K], bf16, kind="ExternalInput")
    w = nc.dram_tensor("weight", [N, K], bf16, kind="ExternalInput")
    bias = nc.dram_tensor("bias", [N], f32, kind="ExternalInput")
    mw = nc.dram_tensor("multiply_weight", [N], f32, kind="ExternalInput")
    if apply_affine:
        gnw = nc.dram_tensor("gn_weight", [N], f32, kind="ExternalInput")
        gnb = nc.dram_tensor("gn_bias", [N], f32, kind="ExternalInput")
    out = nc.dram_tensor("out", [M_SHARD, N], f32, kind="ExternalOutput")

    def bcast_rows(ap_1d, rows):
        # DRAM [n] -> broadcast-read AP [[0, rows], [1, n]]
        return bass.AP(ap_1d.tensor, ap_1d.offset, [[0, rows]] + list(ap_1d.ap))

    with tile.TileContext(nc) as tc:
        from contextlib import ExitStack

        with ExitStack() as ctx:
            xT_pool = ctx.enter_context(
                tc.tile_pool(name="xT", bufs=(1 if M_BLKS == 1 else 2))
            )
            wT_pool = ctx.enter_context(tc.tile_pool(name="wT", bufs=3))
            psum_pool = ctx.enter_context(
                tc.tile_pool(name="psum", bufs=8, space="PSUM")
            )
            big_pool = ctx.enter_context(tc.tile_pool(name="big", bufs=7))
            o_pool = ctx.enter_context(tc.tile_pool(name="o", bufs=2))
            stats_pool = ctx.enter_context(tc.tile_pool(name="stats", bufs=2))
            small_pool = ctx.enter_context(tc.tile_pool(name="small", bufs=2))
            param_pool = ctx.enter_context(tc.tile_pool(name="param", bufs=2))
            const_pool = ctx.enter_context(tc.tile_pool(name="const", bufs=1))

            # ---- constants ----
            magic_f = const_pool.tile([P, 1], f32)
            nc.vector.memset(
                magic_f,
                float(np.frombuffer(np.uint32(0x5F3759DF).tobytes(), np.float32)[0]),
            )
            magic = magic_f.bitcast(i32)
            # fp32r operands keep the bias add at full fp32 precision while
            # running at 1 cycle/row (moving dim 512 >= 256).
            ones_row_f = const_pool.tile([1, P], f32)
            nc.vector.memset(ones_row_f, 1.0)
            ones_row = ones_row_f.bitcast(f32r)

            hwdge = [nc.sync, nc.scalar] if split_queues else [nc.sync, nc.sync]

            # ---- GEMM + epilogue ----
            for mb in range(M_BLKS):
                mrow0 = mb * m_blk
                # x^T tile [P(k), KT, m_blk] -- fully resident
                xT = xT_pool.tile([P, KT, m_blk], bf16, tag="xT")

                def load_xT(kt):
                    hwdge[kt % 2].dma_start_transpose(
                        xT[:, kt, :],
                        x[mrow0 : mrow0 + m_blk, kt * P : (kt + 1) * P],
                    )

                if mb > 0:
                    for kt in range(KT):
                        load_xT(kt)

                def load_ntile(nt, interleave_xt):
                    """Emit wT transpose loads + param loads for n-tile nt.
                    Hoisted one n-tile ahead of its matmuls so the
                    transposes precede the previous epilogue's output
                    stores in the serialized DMA stream (Tile serializes
                    xbar-transpose DMAs against copy DMAs)."""
                    ncol0 = nt * n_tile
                    whs = []
                    for h in range(NKH):
                        wT = wT_pool.tile([P, KH, n_tile], bf16, tag="wT")
                        for j in range(KH):
                            kt = h * KH + j
                            hwdge[j % 2].dma_start_transpose(
                                wT[:, j, :],
                                w[ncol0 : ncol0 + n_tile, kt * P : (kt + 1) * P],
                            )
                            if interleave_xt:
                                load_xT(kt)
                        whs.append(wT)
                    bias_sb = param_pool.tile([1, n_tile], f32r, tag="bias_sb")
                    nc.gpsimd.dma_start(
                        out=bias_sb, in_=bcast_rows(bias[ncol0 : ncol0 + n_tile], 1)
                    )
                    mw_rep = param_pool.tile([P, n_tile], f32, tag="mw_rep")
                    nc.gpsimd.dma_start(
                        out=mw_rep, in_=bcast_rows(mw[ncol0 : ncol0 + n_tile], P)
                    )
                    reps = [None, None]
                    if apply_affine:
                        gnw_rep = param_pool.tile([P, n_tile], f32, tag="gnw_rep")
                        nc.gpsimd.dma_start(
                            out=gnw_rep, in_=bcast_rows(gnw[ncol0 : ncol0 + n_tile], P)
                        )
                        gnb_rep = param_pool.tile([P, n_tile], f32, tag="gnb_rep")
                        nc.gpsimd.dma_start(
                            out=gnb_rep, in_=bcast_rows(gnb[ncol0 : ncol0 + n_tile], P)
                        )
                        reps = [gnw_rep, gnb_rep]
                    return whs, bias_sb, mw_rep, reps

                pending = load_ntile(0, mb == 0)

                for nt in range(N_TILES):
                    ncol0 = nt * n_tile
                    whs, bias_sb, mw_rep, (gnw_rep, gnb_rep) = pending
                    if nt + 1 < N_TILES:
                        pending = load_ntile(nt + 1, False)

                    for grp0 in range(0, MS_PER_BLK, SGRP):
                        ms_list = list(range(grp0, min(grp0 + SGRP, MS_PER_BLK)))
                        pss = {}
                        for ms in ms_list:
                            pss[ms] = psum_pool.tile(
                                [P, n_tile], f32, tag="ps", name=f"ps{ms}"
                            )

                        # k-half waves: all subtiles do kh0, then kh1 --
                        # frees the kh0 wT buffer at ~75% of the n-tile.
                        def emit_mms(ms, h):
                            moff = ms * P
                            for j in range(KH):
                                kt = h * KH + j
                                nc.tensor.matmul(
                                    pss[ms],
                                    lhsT=xT[:, kt, moff : moff + P],
                                    rhs=whs[h][:, j, :],
                                    start=(kt == 0),
                                    stop=False,
                                )
                            if h == NKH - 1:
                                # += ones^T @ bias (adds bias to every row)
                                nc.tensor.matmul(
                                    pss[ms],
                                    lhsT=ones_row[0:1, :],
                                    rhs=bias_sb[0:1, :],
                                    start=False,
                                    stop=True,
                                )

                        if kt_outer:
                            for h in range(NKH):
                                for ms in ms_list:
                                    emit_mms(ms, h)
                        else:
                            for ms in ms_list:
                                for h in range(NKH):
                                    emit_mms(ms, h)

                        for sg0 in range(0, len(ms_list), SG):
                            msl_list = ms_list[sg0 : sg0 + SG]
                            nsg = len(msl_list)
                            sums = stats_pool.tile([P, SG, NG], f32, tag="sums")
                            sqs = stats_pool.tile([P, SG, NG], f32, tag="sqs")
                            t0s = {}
                            for i, ms in enumerate(msl_list):
                                ps = pss[ms]
                                # evacuate PSUM immediately on ACT (frees the
                                # bank for the next wave group's matmuls)
                                t0 = big_pool.tile([P, n_tile], f32, tag="big")
                                nc.scalar.copy(t0, ps)
                                t0s[ms] = t0
                                nc.vector.reduce_sum(
                                    sums[:, i, :],
                                    t0.rearrange("p (g s) -> p g s", s=GS),
                                    axis=X,
                                )
                                sq = big_pool.tile([P, n_tile], f32, tag="big")
                                nc.scalar.square(sq, t0)
                                nc.vector.reduce_sum(
                                    sqs[:, i, :],
                                    sq.rearrange("p (g s) -> p g s", s=GS),
                                    axis=X,
                                )

                            # batched small stats over [P, nsg*NG]
                            sums_f = sums[:, :nsg, :].rearrange("p a b -> p (a b)")
                            sqs_f = sqs[:, :nsg, :].rearrange("p a b -> p (a b)")
                            nb = nsg * NG
                            m2 = small_pool.tile(
                                [P, SG * NG], f32, tag="m2", name="m2"
                            )[:, :nb]
                            nc.vector.tensor_mul(m2, sums_f, sums_f)
                            u = small_pool.tile([P, SG * NG], f32, tag="u", name="u")[
                                :, :nb
                            ]
                            # u = GS*sum(y^2) - sum(y)^2 = GS^2 * var
                            nc.vector.scalar_tensor_tensor(
                                out=u,
                                in0=sqs_f,
                                scalar=float(GS),
                                in1=m2,
                                op0=Alu.mult,
                                op1=Alu.subtract,
                            )
                            nc.vector.tensor_scalar(
                                out=u,
                                in0=u,
                                scalar1=float(GS * GS) * EPS,
                                scalar2=None,
                                op0=Alu.add,
                            )
                            # r = rsqrt(u) = rstd / GS  (Quake + 2 Newton steps)
                            rt = small_pool.tile([P, SG, NG], f32, tag="rt")
                            r = rt[:, :nsg, :].rearrange("p a b -> p (a b)")
                            nc.vector.tensor_scalar(
                                out=r.bitcast(i32),
                                in0=u.bitcast(i32),
                                scalar1=1,
                                scalar2=None,
                                op0=Alu.arith_shift_right,
                            )
                            nc.vector.tensor_tensor(
                                out=r.bitcast(i32),
                                in0=magic.broadcast_to([P, nb]),
                                in1=r.bitcast(i32),
                                op=Alu.subtract,
                            )
                            tnr = small_pool.tile(
                                [P, SG * NG], f32, tag="m2", name="tnr"
                            )[:, :nb]
                            for _ in range(2):
                                nc.vector.tensor_mul(tnr, r, r)
                                nc.vector.tensor_mul(tnr, tnr, u)
                                nc.vector.tensor_scalar(
                                    out=tnr,
                                    in0=tnr,
                                    scalar1=-0.5,
                                    scalar2=1.5,
                                    op0=Alu.mult,
                                    op1=Alu.add,
                                )
                                nc.vector.tensor_mul(r, r, tnr)
                            # U = sum(y) * r = mean * rstd
                            Ut = small_pool.tile([P, SG, NG], f32, tag="m2", name="Ut")
                            nc.vector.tensor_mul(
                                Ut[:, :nsg, :].rearrange("p a b -> p (a b)"),
                                sums_f,
                                r,
                            )

                            for i, ms in enumerate(msl_list):
                                t0 = t0s[ms]
                                t03 = t0.rearrange("p (g s) -> p g s", s=GS)
                                rb = bass.AP(
                                    rt.tensor,
                                    rt[:, i, :].offset,
                                    list(rt[:, i, :].ap) + [[0, GS]],
                                )
                                ub = bass.AP(
                                    Ut.tensor,
                                    Ut[:, i, :].offset,
                                    list(Ut[:, i, :].ap) + [[0, GS]],
                                )
                                t1 = big_pool.tile([P, n_tile], f32, tag="big")
                                t13 = t1.rearrange("p (g s) -> p g s", s=GS)
                                # t1 = (t0 * GS) * r = t0 * rstd
                                nc.vector.scalar_tensor_tensor(
                                    out=t13,
                                    in0=t03,
                                    scalar=float(GS),
                                    in1=rb,
                                    op0=Alu.mult,
                                    op1=Alu.mult,
                                )
                                # t1 -= mean * rstd
                                nc.gpsimd.tensor_tensor(
                                    out=t13, in0=t13, in1=ub, op=Alu.subtract
                                )
                                if apply_affine:
                                    nc.gpsimd.tensor_mul(t1, t1, gnw_rep)
                                    nc.gpsimd.tensor_add(t1, t1, gnb_rep)
                                # swish(v) = v * sigmoid(v); Sigmoid + Square
                                # share one ACT table set (no reloads).
                                s1 = big_pool.tile([P, n_tile], f32, tag="big")
                                nc.scalar.activation(s1, t1, Act.Sigmoid)
                                z = big_pool.tile([P, n_tile], f32, tag="big")
                                nc.vector.tensor_mul(z, s1, t1)
                                nc.gpsimd.tensor_mul(z, z, mw_rep)
                                s2 = big_pool.tile([P, n_tile], f32, tag="big")
                                nc.scalar.activation(s2, z, Act.Sigmoid)
                                o = o_pool.tile([P, n_tile], f32, tag="oo")
                                nc.vector.tensor_mul(o, z, s2)
                                r0 = mrow0 + ms * P
                                nc.gpsimd.dma_start(
                                    out=out[r0 : r0 + P, ncol0 : ncol0 + n_tile],
                                    in_=o,
                                )

    nc.compile()
    return nc


_CACHE = {}


def _get_nc(M_SHARD, K, N, apply_affine):
    key = (M_SHARD, K, N, apply_affine)
    if key not in _CACHE:
        _CACHE[key] = build_nc(M_SHARD, K, N, apply_affine)
    return _CACHE[key]


def make_in_maps(inputs, apply_affine=None):
    import ml_dtypes

    x = np.ascontiguousarray(inputs["x"], dtype=np.float32)
    w = np.ascontiguousarray(inputs["weight"], dtype=np.float32)
    bias = np.ascontiguousarray(inputs["bias"], dtype=np.float32)
    mw = np.ascontiguousarray(inputs["multiply_weight"], dtype=np.float32)
    gnw = np.ascontiguousarray(inputs["gn_weight"], dtype=np.float32)
    gnb = np.ascontiguousarray(inputs["gn_bias"], dtype=np.float32)

    M, K = x.shape
    N = w.shape[0]
    M_SHARD = M // N_CORES
    if apply_affine is None:
        apply_affine = not (np.all(gnw == 1.0) and np.all(gnb == 0.0))

    xb = x.astype(ml_dtypes.bfloat16)
    wb = np.ascontiguousarray(w.astype(ml_dtypes.bfloat16))

    in_maps = []
    for c in range(N_CORES):
        m = {
            "x": np.ascontiguousarray(xb[c * M_SHARD : (c + 1) * M_SHARD]),
            "weight": wb,
            "bias": bias,
            "multiply_weight": mw,
        }
        if apply_affine:
            m["gn_weight"] = gnw
            m["gn_bias"] = gnb
        in_maps.append(m)
    return in_maps, M_SHARD, K, N, apply_affine


def kernel(**inputs):
    from concourse.bass_utils import run_bass_kernel_spmd

    in_maps, M_SHARD, K, N, apply_affine = make_in_maps(inputs)
    nc = _get_nc(M_SHARD, K, N, apply_affine)
    res = run_bass_kernel_spmd(nc, in_maps, core_ids=list(range(N_CORES)))
    return np.concatenate([r["out"] for r in res.results], axis=0)


if __name__ == "__main__":
    import reference

    inputs = {k: np.asarray(v) for k, v in reference.setup_inputs().items()}
    out = kernel(**inputs)
    print(out.shape, out.dtype)


# revision 29
# speedup vs baseline: 1.0414x; 1.0221x over previous
# Fused GEMM + GroupNorm + swish*mw + swish kernel for 8 Trainium2 cores.
#
# reference math (per full problem):
#   y  = x @ W^T + b                      [M, N] = [16384, 4096]
#   yn = GroupNorm(y, groups=256)         (group size 16 along N, eps=1e-6)
#   yn = yn * gn_weight + gn_bias
#   z  = swish(yn) * multiply_weight
#   out= swish(z)
#
# Sharding: data-parallel along M. Each of the 8 cores gets M/8 = 2048 rows of
# x and the full weight/params; outputs are concatenated along M.  The x-shard
# and weight are cast fp32->bf16 host-side (input layout prep); bias is kept
# fp32 on device, PSUM accumulation is fp32, so accuracy matches an
# fp32-accumulated bf16 GEMM.
#
# Per-core schedule:
#   - GEMM operands are loaded K-major via HW xbar DMA-transpose (bf16-only
#     path) on the sync-engine HWDGE queue.  (All transposes stay on ONE
#     queue: concurrent transposes from both HWDGE rings corrupt data on
#     TRN2.)  x^T for the whole 2048-row shard stays resident in SBUF
#     (128KB/partition), so the weight is transpose-read exactly once.
#   - PE: out-tile [128m, 512n] accumulates 32 k-matmuls (bf16 x bf16 -> fp32
#     PSUM) plus one K=1 fp32r matmul of ones^T @ bias_slice that adds the
#     bias at full precision.  The k loop runs in two half-K waves across a
#     group of 8 m-subtiles so each wT half-buffer frees early for prefetch.
#   - GroupNorm stats are reduced straight out of PSUM (DVE sum, ACT square +
#     DVE sum); rstd uses a Quake-style rsqrt (bit trick + 2 Newton steps) on
#     DVE, batched over 4 m-subtiles, avoiding ACT table swaps.
#   - normalize on DVE (scalar_tensor_tensor), mean-sub + multiply_weight on
#     GpSimd, swish = x*sigmoid(x) with Sigmoid on ACT and multiplies on DVE,
#     output stores on the GpSimd (SWDGE) queue.

import numpy as np

P = 128
GS = 16  # group size = N / NUM_GROUPS = 4096 / 256
EPS = 1e-6

M_FULL, K_FULL, N_FULL = 16384, 4096, 4096
N_CORES = 8


def build_nc(M_SHARD, K, N, apply_affine, n_tile=512, m_blk=2048,
             split_queues=False, kt_outer=True):
    import concourse.bass as bass
    import concourse.tile as tile
    from concourse import bacc, mybir

    f32 = mybir.dt.float32
    f32r = mybir.dt.float32r
    bf16 = mybir.dt.bfloat16
    i32 = mybir.dt.int32
    Alu = mybir.AluOpType
    Act = mybir.ActivationFunctionType
    X = mybir.AxisListType.X

    KT = K // P                    # k-tiles of 128
    KH = max(KT // 2, 1)           # k-tiles per wT half-buffer
    NKH = KT // KH                 # number of k-half waves (2)
    N_TILES = N // n_tile
    NG = n_tile // GS              # groups per n-tile
    m_blk = min(m_blk, M_SHARD)
    M_BLKS = M_SHARD // m_blk
    MS_PER_BLK = m_blk // P
    SGRP = min(8, MS_PER_BLK)      # m-subtiles per PSUM wave group
    SG = min(4, SGRP)              # m-subtiles per stats batch

    nc = bacc.Bacc("TRN2", target_bir_lowering=False)

    x = nc.dram_tensor("x", [M_SHARD, K], bf16, kind="ExternalInput")
    w = nc.dram_tensor("weight", [N, K], bf16, kind="ExternalInput")
    bias = nc.dram_tensor("bias", [N], f32, kind="ExternalInput")
    mw = nc.dram_tensor("multiply_weight", [N], f32, kind="ExternalInput")
    if apply_affine:
        gnw = nc.dram_tensor("gn_weight", [N], f32, kind="ExternalInput")
        gnb = nc.dram_tensor("gn_bias", [N], f32, kind="ExternalInput")
    out = nc.dram_tensor("out", [M_SHARD, N], f32, kind="ExternalOutput")

    def bcast_rows(ap_1d, rows):
        # DRAM [n] -> broadcast-read AP [[0, rows], [1, n]]
        return bass.AP(ap_1d.tensor, ap_1d.offset, [[0, rows]] + list(ap_1d.ap))

    with tile.TileContext(nc) as tc:
        from contextlib import ExitStack

        with ExitStack() as ctx:
            xT_pool = ctx.enter_context(
                tc.tile_pool(name="xT", bufs=(1 if M_BLKS == 1 else 2))
            )
            wT_pool = ctx.enter_context(tc.tile_pool(name="wT", bufs=2))
            psum_pool = ctx.enter_context(
                tc.tile_pool(name="psum", bufs=8, space="PSUM")
            )
            big_pool = ctx.enter_context(tc.tile_pool(name="big", bufs=14))
            o_pool = ctx.enter_context(tc.tile_pool(name="o", bufs=2))
            stats_pool = ctx.enter_context(tc.tile_pool(name="stats", bufs=2))
            small_pool = ctx.enter_context(tc.tile_pool(name="small", bufs=2))
            param_pool = ctx.enter_context(tc.tile_pool(name="param", bufs=2))
            const_pool = ctx.enter_context(tc.tile_pool(name="const", bufs=1))

            # ---- constants ----
            magic_f = const_pool.tile([P, 1], f32)
            nc.vector.memset(
                magic_f,
                float(np.frombuffer(np.uint32(0x5F3759DF).tobytes(), np.float32)[0]),
            )
            magic = magic_f.bitcast(i32)
            # fp32r operands keep the bias add at full fp32 precision while
            # running at 1 cycle/row (moving dim 512 >= 256).
            ones_row_f = const_pool.tile([1, P], f32)
            nc.vector.memset(ones_row_f, 1.0)
            ones_row = ones_row_f.bitcast(f32r)

            hwdge = [nc.sync, nc.scalar] if split_queues else [nc.sync, nc.sync]

            # ---- GEMM + epilogue ----
            for mb in range(M_BLKS):
                mrow0 = mb * m_blk
                # x^T tile [P(k), KT, m_blk] -- fully resident
                xT = xT_pool.tile([P, KT, m_blk], bf16, tag="xT")

                def load_xT(kt):
                    hwdge[kt % 2].dma_start_transpose(
                        xT[:, kt, :],
                        x[mrow0 : mrow0 + m_blk, kt * P : (kt + 1) * P],
                    )

                if mb > 0:
                    for kt in range(KT):
                        load_xT(kt)

                def load_ntile(nt, interleave_xt):
                    """Emit wT transpose loads + param loads for n-tile nt.
                    Hoisted one n-tile ahead of its matmuls so the
                    transposes precede the previous epilogue's output
                    stores in the serialized DMA stream (Tile serializes
                    xbar-transpose DMAs against copy DMAs)."""
                    ncol0 = nt * n_tile
                    whs = []
                    for h in range(NKH):
                        wT = wT_pool.tile([P, KH, n_tile], bf16, tag="wT")
                        for j in range(KH):
                            kt = h * KH + j
                            hwdge[j % 2].dma_start_transpose(
                                wT[:, j, :],
                                w[ncol0 : ncol0 + n_tile, kt * P : (kt + 1) * P],
                            )
                            if interleave_xt:
                                load_xT(kt)
                        whs.append(wT)
                    bias_sb = param_pool.tile([1, n_tile], f32r, tag="bias_sb")
                    nc.gpsimd.dma_start(
                        out=bias_sb, in_=bcast_rows(bias[ncol0 : ncol0 + n_tile], 1)
                    )
                    mw_rep = param_pool.tile([P, n_tile], f32, tag="mw_rep")
                    nc.gpsimd.dma_start(
                        out=mw_rep, in_=bcast_rows(mw[ncol0 : ncol0 + n_tile], P)
                    )
                    reps = [None, None]
                    if apply_affine:
                        gnw_rep = param_pool.tile([P, n_tile], f32, tag="gnw_rep")
                        nc.gpsimd.dma_start(
                            out=gnw_rep, in_=bcast_rows(gnw[ncol0 : ncol0 + n_tile], P)
                        )
                        gnb_rep = param_pool.tile([P, n_tile], f32, tag="gnb_rep")
                        nc.gpsimd.dma_start(
                            out=gnb_rep, in_=bcast_rows(gnb[ncol0 : ncol0 + n_tile], P)
                        )
                        reps = [gnw_rep, gnb_rep]
                    return whs, bias_sb, mw_rep, reps

                pending = load_ntile(0, mb == 0)

                for nt in range(N_TILES):
                    ncol0 = nt * n_tile
                    whs, bias_sb, mw_rep, (gnw_rep, gnb_rep) = pending
                    if nt + 1 < N_TILES:
                        pending = load_ntile(nt + 1, False)

                    for grp0 in range(0, MS_PER_BLK, SGRP):
                        ms_list = list(range(grp0, min(grp0 + SGRP, MS_PER_BLK)))
                        pss = {}
                        for ms in ms_list:
                            pss[ms] = psum_pool.tile(
                                [P, n_tile], f32, tag="ps", name=f"ps{ms}"
                            )

                        # k-half waves: all subtiles do kh0, then kh1 --
                        # frees the kh0 wT buffer at ~75% of the n-tile.
                        def emit_mms(ms, h):
                            moff = ms * P
                            for j in range(KH):
                                kt = h * KH + j
                                nc.tensor.matmul(
                                    pss[ms],
                                    lhsT=xT[:, kt, moff : moff + P],
                                    rhs=whs[h][:, j, :],
                                    start=(kt == 0),
                                    stop=False,
                                )
                            if h == NKH - 1:
                                # += ones^T @ bias (adds bias to every row)
                                nc.tensor.matmul(
                                    pss[ms],
                                    lhsT=ones_row[0:1, :],
                                    rhs=bias_sb[0:1, :],
                                    start=False,
                                    stop=True,
                                )

                        if kt_outer:
                            for h in range(NKH):
                                for ms in ms_list:
                                    emit_mms(ms, h)
                        else:
                            for ms in ms_list:
                                for h in range(NKH):
                                    emit_mms(ms, h)

                        # evacuate PSUM immediately on ACT, all subtiles
                        # back-to-back right after the group's matmuls, so the
                        # banks free before the ACT queue dives into the
                        # previous group's sigmoids
                        t0s = {}
                        for ms in ms_list:
                            t0 = big_pool.tile([P, n_tile], f32, tag="big", name=f"t0_{ms}")
                            nc.scalar.copy(t0, pss[ms])
                            t0s[ms] = t0

                        for sg0 in range(0, len(ms_list), SG):
                            msl_list = ms_list[sg0 : sg0 + SG]
                            nsg = len(msl_list)
                            sums = stats_pool.tile([P, SG, NG], f32, tag="sums")
                            sqs = stats_pool.tile([P, SG, NG], f32, tag="sqs")
                            for i, ms in enumerate(msl_list):
                                t0 = t0s[ms]
                                nc.vector.reduce_sum(
                                    sums[:, i, :],
                                    t0.rearrange("p (g s) -> p g s", s=GS),
                                    axis=X,
                                )
                                sq = big_pool.tile([P, n_tile], f32, tag="big")
                                nc.scalar.square(sq, t0)
                                nc.vector.reduce_sum(
                                    sqs[:, i, :],
                                    sq.rearrange("p (g s) -> p g s", s=GS),
                                    axis=X,
                                )

                            # batched small stats over [P, nsg*NG]
                            sums_f = sums[:, :nsg, :].rearrange("p a b -> p (a b)")
                            sqs_f = sqs[:, :nsg, :].rearrange("p a b -> p (a b)")
                            nb = nsg * NG
                            m2 = small_pool.tile(
                                [P, SG * NG], f32, tag="m2", name="m2"
                            )[:, :nb]
                            nc.vector.tensor_mul(m2, sums_f, sums_f)
                            u = small_pool.tile([P, SG * NG], f32, tag="u", name="u")[
                                :, :nb
                            ]
                            # u = GS*sum(y^2) - sum(y)^2 = GS^2 * var
                            nc.vector.scalar_tensor_tensor(
                                out=u,
                                in0=sqs_f,
                                scalar=float(GS),
                                in1=m2,
                                op0=Alu.mult,
                                op1=Alu.subtract,
                            )
                            nc.vector.tensor_scalar(
                                out=u,
                                in0=u,
                                scalar1=float(GS * GS) * EPS,
                                scalar2=None,
                                op0=Alu.add,
                            )
                            # r = rsqrt(u) = rstd / GS  (Quake + 2 Newton steps)
                            rt = small_pool.tile([P, SG, NG], f32, tag="rt")
                            r = rt[:, :nsg, :].rearrange("p a b -> p (a b)")
                            nc.vector.tensor_scalar(
                                out=r.bitcast(i32),
                                in0=u.bitcast(i32),
                                scalar1=1,
                                scalar2=None,
                                op0=Alu.arith_shift_right,
                            )
                            nc.vector.tensor_tensor(
                                out=r.bitcast(i32),
                                in0=magic.broadcast_to([P, nb]),
                                in1=r.bitcast(i32),
                                op=Alu.subtract,
                            )
                            tnr = small_pool.tile(
                                [P, SG * NG], f32, tag="m2", name="tnr"
                            )[:, :nb]
                            for _ in range(2):
                                nc.vector.tensor_mul(tnr, r, r)
                                nc.vector.tensor_mul(tnr, tnr, u)
                                nc.vector.tensor_scalar(
                                    out=tnr,
                                    in0=tnr,
                                    scalar1=-0.5,
                                    scalar2=1.5,
                                    op0=Alu.mult,
                                    op1=Alu.add,
                                )
                                nc.vector.tensor_mul(r, r, tnr)
                            # U = sum(y) * r = mean * rstd
                            Ut = small_pool.tile([P, SG, NG], f32, tag="m2", name="Ut")
                            nc.vector.tensor_mul(
                                Ut[:, :nsg, :].rearrange("p a b -> p (a b)"),
                                sums_f,
                                r,
                            )

                            for i, ms in enumerate(msl_list):
                                t0 = t0s[ms]
                                t03 = t0.rearrange("p (g s) -> p g s", s=GS)
                                rb = bass.AP(
                                    rt.tensor,
                                    rt[:, i, :].offset,
                                    list(rt[:, i, :].ap) + [[0, GS]],
                                )
                                ub = bass.AP(
                                    Ut.tensor,
                                    Ut[:, i, :].offset,
                                    list(Ut[:, i, :].ap) + [[0, GS]],
                                )
                                t1 = big_pool.tile([P, n_tile], f32, tag="big")
                                t13 = t1.rearrange("p (g s) -> p g s", s=GS)
                                # t1 = (t0 * GS) * r = t0 * rstd
                                nc.vector.scalar_tensor_tensor(
                                    out=t13,
                                    in0=t03,
                                    scalar=float(GS),
                                    in1=rb,
                                    op0=Alu.mult,
                                    op1=Alu.mult,
                                )
                                # t1 -= mean * rstd
                                nc.gpsimd.tensor_tensor(
                                    out=t13, in0=t13, in1=ub, op=Alu.subtract
                                )
                                if apply_affine:
                                    nc.gpsimd.tensor_mul(t1, t1, gnw_rep)
                                    nc.gpsimd.tensor_add(t1, t1, gnb_rep)
                                # swish(v) = v * sigmoid(v); Sigmoid + Square
                                # share one ACT table set (no reloads).
                                s1 = big_pool.tile([P, n_tile], f32, tag="big")
                                nc.scalar.activation(s1, t1, Act.Sigmoid)
                                z = big_pool.tile([P, n_tile], f32, tag="big")
                                nc.vector.tensor_mul(z, s1, t1)
                                nc.gpsimd.tensor_mul(z, z, mw_rep)
                                s2 = big_pool.tile([P, n_tile], f32, tag="big")
                                nc.scalar.activation(s2, z, Act.Sigmoid)
                                o = o_pool.tile([P, n_tile], f32, tag="oo")
                                nc.vector.tensor_mul(o, z, s2)
                                r0 = mrow0 + ms * P
                                nc.gpsimd.dma_start(
                                    out=out[r0 : r0 + P, ncol0 : ncol0 + n_tile],
                                    in_=o,
                                )

    nc.compile()
    return nc


_CACHE = {}


def _get_nc(M_SHARD, K, N, apply_affine):
    key = (M_SHARD, K, N, apply_affine)
    if key not in _CACHE:
        _CACHE[key] = build_nc(M_SHARD, K, N, apply_affine)
    return _CACHE[key]


def make_in_maps(inputs, apply_affine=None):
    import ml_dtypes

    x = np.ascontiguousarray(inputs["x"], dtype=np.float32)
    w = np.ascontiguousarray(inputs["weight"], dtype=np.float32)
    bias = np.ascontiguousarray(inputs["bias"], dtype=np.float32)
    mw = np.ascontiguousarray(inputs["multiply_weight"], dtype=np.float32)
    gnw = np.ascontiguousarray(inputs["gn_weight"], dtype=np.float32)
    gnb = np.ascontiguousarray(inputs["gn_bias"], dtype=np.float32)

    M, K = x.shape
    N = w.shape[0]
    M_SHARD = M // N_CORES
    if apply_affine is None:
        apply_affine = not (np.all(gnw == 1.0) and np.all(gnb == 0.0))

    xb = x.astype(ml_dtypes.bfloat16)
    wb = np.ascontiguousarray(w.astype(ml_dtypes.bfloat16))

    in_maps = []
    for c in range(N_CORES):
        m = {
            "x": np.ascontiguousarray(xb[c * M_SHARD : (c + 1) * M_SHARD]),
            "weight": wb,
            "bias": bias,
            "multiply_weight": mw,
        }
        if apply_affine:
            m["gn_weight"] = gnw
            m["gn_bias"] = gnb
        in_maps.append(m)
    return in_maps, M_SHARD, K, N, apply_affine


def kernel(**inputs):
    from concourse.bass_utils import run_bass_kernel_spmd

    in_maps, M_SHARD, K, N, apply_affine = make_in_maps(inputs)
    nc = _get_nc(M_SHARD, K, N, apply_affine)
    res = run_bass_kernel_spmd(nc, in_maps, core_ids=list(range(N_CORES)))
    return np.concatenate([r["out"] for r in res.results], axis=0)


if __name__ == "__main__":
    import reference

    inputs = {k: np.asarray(v) for k, v in reference.setup_inputs().items()}
    out = kernel(**inputs)
    print(out.shape, out.dtype)
